# revision 14
# baseline (speedup 1.0000x reference)
"""Multi-head causal attention (B=2, T=2048, C=1024, H=16, D=64) on 8 TRN2
NeuronCores, tensor-parallel over heads: core c owns heads {2c, 2c+1}.

Per-core program (SPMD, same code, per-core weight slices), all matmuls bf16
with fp32 PSUM accumulation. Pipelined in 4 stages of 1024 tokens each:
projection chunk -> V-transposes + zero-padded K^T piece -> the attention
q-chunks whose causal window is now complete. This keeps TensorE dense
(HAM stays at full clock) and lets ScalarE exp overlap the next projection.

Attention per (b, head, 512-wide q-chunk):
  s^T[k, q] = Kpad^T.T @ Q^T  (K=128 via zero-padding, full-array matmuls;
  two k-tiles share a 2-bank psum -> one 1024-wide exp), causal diagonal
  tiles get range-limited exp + gpsimd affine_select zeroing,
  outT[d,q] + denom row = [V | 1].T @ P^T with M=128 overlapping lhsT
  windows, PE-transpose back to [q, d] (bf16), scale by 1/denom on DVE.
Host: transposes x, converts x/W to bf16, slices W/b per core, concatenates
per-core [4096, 128] outputs on channels.
"""
import sys

if "/opt/trn_rl_repo" not in sys.path:
    sys.path.insert(0, "/opt/trn_rl_repo")

from contextlib import ExitStack

import numpy as np
import ml_dtypes

import concourse.bass as bass
import concourse.tile as tile
from concourse import bacc, mybir
from concourse._compat import with_exitstack
from concourse.bass_utils import run_bass_kernel_spmd
from concourse.masks import make_identity

F32 = mybir.dt.float32
BF16 = mybir.dt.bfloat16
EXPF = mybir.ActivationFunctionType.Exp

B, T, C = 2, 2048, 1024
H, D = 16, 64
NCORES = 8
TOK = B * T            # 4096
CS = 128               # channel slice per core (2 heads x 64)
N3 = 3 * CS            # 384 qkv columns per core
SCALE = 1.0 / np.sqrt(D)


@with_exitstack
def mha_kernel(ctx: ExitStack, tc: tile.TileContext, out_ap, xT_ap, w_ap, b_ap):
    nc = tc.nc

    cst = ctx.enter_context(tc.tile_pool(name="cst", bufs=1))
    xpool = ctx.enter_context(tc.tile_pool(name="xt", bufs=10))
    pbig = ctx.enter_context(tc.tile_pool(name="pbig", bufs=10))
    pdiag = ctx.enter_context(tc.tile_pool(name="pdiag", bufs=8))
    otpool = ctx.enter_context(tc.tile_pool(name="ot", bufs=3))
    rpool = ctx.enter_context(tc.tile_pool(name="rc", bufs=3))
    psA = ctx.enter_context(tc.tile_pool(name="psA", bufs=2, space="PSUM"))  # big: proj + paired scores
    psB = ctx.enter_context(tc.tile_pool(name="psB", bufs=2, space="PSUM"))  # diag scores
    psC = ctx.enter_context(tc.tile_pool(name="psC", bufs=1, space="PSUM"))  # pv
    psD = ctx.enter_context(tc.tile_pool(name="psD", bufs=1, space="PSUM"))  # transposes

    ident_f = cst.tile([128, 128], F32, name="ident_f")
    make_identity(nc, ident_f[:])
    ident_b = cst.tile([128, 128], BF16, name="ident_b")
    nc.vector.tensor_copy(ident_b[:], ident_f[:])

    # bias cols 0-2; cols 3-18 are ones (for the PV denominator column)
    bias = cst.tile([128, 19], F32, name="bias")
    nc.sync.dma_start(bias[:], b_ap[:])

    W8 = []
    for cc in range(8):
        w = cst.tile([128, N3], BF16, name=f"w{cc}")
        nc.sync.dma_start(w[:], w_ap[128 * cc : 128 * (cc + 1), :])
        W8.append(w)

    qkvT = [cst.tile([128, TOK], BF16, name=f"qkvT{nt}") for nt in range(3)]

    # persistent attention tensors
    # kpad[b][0]: rows 0-63 = K^T_h0, rows 64-127 = 0
    # kpad[b][1]: rows 0-63 = 0,      rows 64-127 = K^T_h1
    kpad = []
    for b in range(B):
        k0t = cst.tile([128, 2048], BF16, name=f"kpad0_{b}")
        nc.vector.memset(k0t[64:128, :], 0.0)
        k1t = cst.tile([128, 2048], BF16, name=f"kpad1_{b}")
        nc.vector.memset(k1t[0:64, :], 0.0)
        kpad.append((k0t, k1t))
    V1 = []
    for b in range(B):
        for hh in range(2):
            v1 = cst.tile([128, 16 * 65], BF16, name=f"v1_{b}_{hh}")
            v3 = v1[:].rearrange("p (k u) -> p k u", u=65)
            nc.vector.tensor_copy(v3[:, :, 64], bias[:, 3:19])
            V1.append(v1)
    outsb = [cst.tile([128, 2048], F32, name=f"outsb{b}") for b in range(B)]

    def attention_unit(b, hh, qc):
        q0 = 2048 * b + 512 * qc
        nk = 4 * qc + 4
        kh = kpad[b][hh]
        qh = qkvT[0][:, q0 : q0 + 512]
        v1 = V1[2 * b + hh]

        # paired full k-tiles -> [128, 1024] psum -> one wide exp
        rhs_slices = []  # (P tile, col offset or diag marker) per ki
        for kj in range(2 * qc):
            spb = psA.tile([128, 1024], F32, name="psA")
            for half in range(2):
                ki = 2 * kj + half
                nc.tensor.matmul(
                    spb[:, 512 * half : 512 * (half + 1)],
                    lhsT=kh[:, 128 * ki : 128 * (ki + 1)],
                    rhs=qh,
                    start=True,
                    stop=True,
                )
            Pb = pbig.tile([128, 1024], BF16, name="Pbig")
            nc.scalar.activation(Pb[:], spb[:], EXPF, scale=SCALE)
            rhs_slices.append((Pb, 0))
            rhs_slices.append((Pb, 512))
        # diagonal k-tiles
        for ki in range(4 * qc, nk):
            spd = psB.tile([128, 512], F32, name="psB")
            nc.tensor.matmul(
                spd[:],
                lhsT=kh[:, 128 * ki : 128 * (ki + 1)],
                rhs=qh,
                start=True,
                stop=True,
            )
            Pd = pdiag.tile([128, 512], BF16, name="Pdiag")
            r = 128 * ki - 512 * qc
            nc.scalar.activation(Pd[:, r:512], spd[:, r:512], EXPF, scale=SCALE)
            nc.gpsimd.affine_select(
                out=Pd[:, r:512],
                in_=Pd[:, r:512],
                compare_op=mybir.AluOpType.is_ge,
                fill=0.0,
                base=0,
                pattern=[[1, 512 - r]],
                channel_multiplier=-1,
            )
            rhs_slices.append((Pd, r - 512))  # negative marks diag offset r

        op = psC.tile([128, 512], F32, name="psC")
        for ki in range(nk):
            Pt, off = rhs_slices[ki]
            if off >= 0:
                rhs = Pt[:, off : off + 512]
                r = 0
            else:
                r = off + 512
                rhs = Pt[:, r:512]
            # M=128 windows read 63 cols into chunk ki+1; the last diag of an
            # odd qc is produced this same stage, so stop at the chunk edge.
            m = 65 if (ki == nk - 1 and qc % 2 == 1) else 128
            nc.tensor.matmul(
                op[0:m, r:512],
                lhsT=v1[:, 65 * ki : 65 * ki + m],
                rhs=rhs,
                start=(ki == 0),
                stop=(ki == nk - 1),
                skip_group_check=True,
            )
        oT = otpool.tile([65, 512], BF16, name="oT")
        nc.vector.tensor_copy(oT[:], op[0:65, :])

        tr = psD.tile([128, 272], BF16, name="psD_tr", tag="psD")
        for j in range(4):
            nc.tensor.transpose(
                tr[:, 256 + 4 * j : 257 + 4 * j],
                oT[64:65, 128 * j : 128 * (j + 1)],
                ident_b[64:65, 64:65],
            )
            nc.tensor.transpose(
                tr[:, 64 * j : 64 * (j + 1)],
                oT[0:64, 128 * j : 128 * (j + 1)],
                ident_b[0:64, 0:64],
            )
        rc = rpool.tile([128, 4], F32, name="rc")
        den4 = tr[:, 256:272].rearrange("p (a b) -> p a b", b=4)[:, :, 0]
        nc.vector.reciprocal(rc[:], den4)
        for j in range(4):
            col = 128 * (4 * qc + j) + 64 * hh
            nc.vector.tensor_scalar_mul(
                outsb[b][:, col : col + 64],
                tr[:, 64 * j : 64 * (j + 1)],
                rc[:, j : j + 1],
            )

    # ---- pipelined stages: proj chunk -> K/V prep piece -> ready q-chunks ----
    for tq in range(4):  # 1024-token stages; b = tq // 2, piece g = tq % 2
        b, g = tq // 2, tq % 2
        xts = []
        for cc in range(8):
            xt = xpool.tile([128, 1024], BF16, name="xt")
            for hv in range(2):
                nc.sync.dma_start(
                    xt[:, 512 * hv : 512 * (hv + 1)],
                    xT_ap[128 * cc : 128 * (cc + 1),
                          1024 * tq + 512 * hv : 1024 * tq + 512 * (hv + 1)],
                )
            xts.append(xt)
        for th in range(2):
            t0 = 1024 * tq + 512 * th
            for nt in range(3):
                ps = psA.tile([128, 1024], F32, name="psA")
                for cc in range(8):
                    nc.tensor.matmul(
                        ps[:, 0:512],
                        lhsT=W8[cc][:, 128 * nt : 128 * (nt + 1)],
                        rhs=xts[cc][:, 512 * th : 512 * (th + 1)],
                        start=(cc == 0),
                        stop=(cc == 7),
                    )
                nc.vector.tensor_scalar_add(
                    qkvT[nt][:, t0 : t0 + 512], ps[:, 0:512], bias[:, nt : nt + 1]
                )

        # K^T zero-padded piece (this stage's 1024 tokens)
        lo = 1024 * g
        nc.vector.tensor_copy(
            kpad[b][0][0:64, lo : lo + 1024],
            qkvT[1][0:64, 1024 * tq : 1024 * (tq + 1)],
        )
        nc.vector.tensor_copy(
            kpad[b][1][64:128, lo : lo + 1024],
            qkvT[1][64:128, 1024 * tq : 1024 * (tq + 1)],
        )
        # V natural piece: k-chunks 8g .. 8g+7
        for hh in range(2):
            v3 = V1[2 * b + hh][:].rearrange("p (k u) -> p k u", u=65)
            tp = psD.tile([128, 512], BF16, name="psD_vt", tag="psD")
            for j in range(8):
                kc = 8 * g + j
                nc.tensor.transpose(
                    tp[:, 64 * j : 64 * (j + 1)],
                    qkvT[2][64 * hh : 64 * (hh + 1), 2048 * b + 128 * kc : 2048 * b + 128 * (kc + 1)],
                    ident_b[64 * hh : 64 * hh + 64, 64 * hh : 64 * hh + 64],
                )
            nc.vector.tensor_copy(
                v3[:, 8 * g : 8 * (g + 1), 0:64],
                tp[:].rearrange("p (k u) -> p k u", u=64),
            )

        # attention q-chunks now complete: qc in {2g, 2g+1}; long one first
        for qc in (2 * g + 1, 2 * g):
            for hh in range(2):
                attention_unit(b, hh, qc)
            # this (b, qc) row block is complete: stream it out
            dst = out_ap[2048 * b + 512 * qc : 2048 * b + 512 * (qc + 1), :]
            dst = dst.rearrange("(k p) c -> p k c", p=128)
            src = outsb[b][:, 512 * qc : 512 * (qc + 1)]
            src = src.rearrange("p (k c) -> p k c", c=128)
            nc.sync.dma_start(dst, src)


def build_program():
    nc = bacc.Bacc("TRN2", target_bir_lowering=False, debug=False, num_devices=NCORES)
    xT_h = nc.dram_tensor("xT", [C, TOK], BF16, kind="ExternalInput").ap()
    w_h = nc.dram_tensor("w", [C, N3], BF16, kind="ExternalInput").ap()
    b_h = nc.dram_tensor("b", [128, 19], F32, kind="ExternalInput").ap()
    out_h = nc.dram_tensor("out", [TOK, CS], F32, kind="ExternalOutput").ap()
    with tile.TileContext(nc) as tc:
        mha_kernel(tc, out_h, xT_h, w_h, b_h)
    nc.compile()
    return nc


def make_in_maps(x, w_qkv, b_qkv):
    x = np.asarray(x, dtype=np.float32)
    w_qkv = np.asarray(w_qkv, dtype=np.float32)
    b_qkv = np.asarray(b_qkv, dtype=np.float32)
    xT = np.ascontiguousarray(x.reshape(TOK, C).T).astype(ml_dtypes.bfloat16)
    in_maps = []
    for c in range(NCORES):
        sl = slice(CS * c, CS * (c + 1))
        w_c = np.ascontiguousarray(
            np.concatenate(
                [w_qkv[:, sl], w_qkv[:, C + CS * c : C + CS * (c + 1)],
                 w_qkv[:, 2 * C + CS * c : 2 * C + CS * (c + 1)]],
                axis=1,
            )
        ).astype(ml_dtypes.bfloat16)
        b_c = np.concatenate(
            [b_qkv[sl], b_qkv[C + CS * c : C + CS * (c + 1)],
             b_qkv[2 * C + CS * c : 2 * C + CS * (c + 1)]]
        )
        b_c = b_c.reshape(3, 128).T  # [128, 3]
        b_c = np.ascontiguousarray(
            np.concatenate([b_c, np.ones((128, 16), np.float32)], axis=1)
        )  # [128, 19]
        in_maps.append({"xT": xT, "w": w_c, "b": b_c})
    return in_maps


_NC_CACHE = None


def kernel(x, w_qkv, b_qkv):
    global _NC_CACHE
    if _NC_CACHE is None:
        _NC_CACHE = build_program()
    nc = _NC_CACHE
    in_maps = make_in_maps(x, w_qkv, b_qkv)
    res = run_bass_kernel_spmd(nc, in_maps, list(range(NCORES)))
    outs = [res.results[c]["out"].reshape(B, T, CS) for c in range(NCORES)]
    return np.concatenate(outs, axis=2)


if __name__ == "__main__":
    rng = np.random.default_rng(0)
    x = rng.standard_normal((B, T, C), dtype=np.float32)
    w = (rng.standard_normal((C, 3 * C), dtype=np.float32) / np.sqrt(C)).astype(np.float32)
    bq = (rng.standard_normal((3 * C,), dtype=np.float32) * 0.02).astype(np.float32)
    out = kernel(x, w, bq)
    print("out", out.shape, out.dtype)


# revision 15
# speedup vs baseline: 76.4528x; 76.4528x over previous
"""Multi-head causal attention (B=2, T=2048, C=1024, H=16, D=64) on 8 TRN2
NeuronCores, tensor-parallel over heads: core c owns heads {2c, 2c+1}.

Per-core program (SPMD, same code, per-core weight slices), all matmuls bf16
with fp32 PSUM accumulation. Pipelined in 4 stages of 1024 tokens each:
projection chunk -> V-transposes + zero-padded K^T piece -> the attention
q-chunks whose causal window is now complete. This keeps TensorE dense
(HAM stays at full clock) and lets ScalarE exp overlap the next projection.

Attention per (b, head, 512-wide q-chunk):
  s^T[k, q] = Kpad^T.T @ Q^T  (K=128 via zero-padding, full-array matmuls;
  two k-tiles share a 2-bank psum -> one 1024-wide exp), causal diagonal
  tiles get range-limited exp + gpsimd affine_select zeroing,
  outT[d,q] + denom row = [V | 1].T @ P^T with M=128 overlapping lhsT
  windows, PE-transpose back to [q, d] (bf16), scale by 1/denom on DVE.
Host: transposes x, converts x/W to bf16, slices W/b per core, concatenates
per-core [4096, 128] outputs on channels.
"""
import sys

if "/opt/trn_rl_repo" not in sys.path:
    sys.path.insert(0, "/opt/trn_rl_repo")

from contextlib import ExitStack

import numpy as np
import ml_dtypes

import concourse.bass as bass
import concourse.tile as tile
from concourse import bacc, mybir
from concourse._compat import with_exitstack
from concourse.bass_utils import run_bass_kernel_spmd
from concourse.masks import make_identity

F32 = mybir.dt.float32
BF16 = mybir.dt.bfloat16
EXPF = mybir.ActivationFunctionType.Exp

B, T, C = 2, 2048, 1024
H, D = 16, 64
NCORES = 8
TOK = B * T            # 4096
CS = 128               # channel slice per core (2 heads x 64)
N3 = 3 * CS            # 384 qkv columns per core
SCALE = 1.0 / np.sqrt(D)


@with_exitstack
def mha_kernel(ctx: ExitStack, tc: tile.TileContext, out_ap, xT_ap, w_ap, b_ap):
    nc = tc.nc

    cst = ctx.enter_context(tc.tile_pool(name="cst", bufs=1))
    xpool = ctx.enter_context(tc.tile_pool(name="xt", bufs=10))
    pbig = ctx.enter_context(tc.tile_pool(name="pbig", bufs=10))
    pdiag = ctx.enter_context(tc.tile_pool(name="pdiag", bufs=8))
    otpool = ctx.enter_context(tc.tile_pool(name="ot", bufs=3))
    rpool = ctx.enter_context(tc.tile_pool(name="rc", bufs=3))
    psA = ctx.enter_context(tc.tile_pool(name="psA", bufs=2, space="PSUM"))  # big: proj + paired scores
    psB = ctx.enter_context(tc.tile_pool(name="psB", bufs=2, space="PSUM"))  # diag scores
    psC = ctx.enter_context(tc.tile_pool(name="psC", bufs=1, space="PSUM"))  # pv
    psD = ctx.enter_context(tc.tile_pool(name="psD", bufs=1, space="PSUM"))  # transposes

    ident_f = cst.tile([128, 128], F32, name="ident_f")
    make_identity(nc, ident_f[:])
    ident_b = cst.tile([128, 128], BF16, name="ident_b")
    nc.vector.tensor_copy(ident_b[:], ident_f[:])

    # bias cols 0-2; cols 3-18 are ones (for the PV denominator column)
    bias = cst.tile([128, 19], F32, name="bias")
    nc.sync.dma_start(bias[:], b_ap[:])

    W8 = []
    for cc in range(8):
        w = cst.tile([128, N3], BF16, name=f"w{cc}")
        nc.sync.dma_start(w[:], w_ap[128 * cc : 128 * (cc + 1), :])
        W8.append(w)

    qkvT = [cst.tile([128, TOK], BF16, name=f"qkvT{nt}") for nt in range(3)]

    # persistent attention tensors
    # kpad[b][0]: rows 0-63 = K^T_h0, rows 64-127 = 0
    # kpad[b][1]: rows 0-63 = 0,      rows 64-127 = K^T_h1
    kpad = []
    for b in range(B):
        k0t = cst.tile([128, 2048], BF16, name=f"kpad0_{b}")
        nc.vector.memset(k0t[64:128, :], 0.0)
        k1t = cst.tile([128, 2048], BF16, name=f"kpad1_{b}")
        nc.vector.memset(k1t[0:64, :], 0.0)
        kpad.append((k0t, k1t))
    V1 = []
    for b in range(B):
        for hh in range(2):
            v1 = cst.tile([128, 16 * 65], BF16, name=f"v1_{b}_{hh}")
            v3 = v1[:].rearrange("p (k u) -> p k u", u=65)
            nc.vector.tensor_copy(v3[:, :, 64], bias[:, 3:19])
            V1.append(v1)
    outsb = [cst.tile([128, 2048], F32, name=f"outsb{b}") for b in range(B)]

    def attention_unit(b, hh, qc):
        q0 = 2048 * b + 512 * qc
        nk = 4 * qc + 4
        kh = kpad[b][hh]
        qh = qkvT[0][:, q0 : q0 + 512]
        v1 = V1[2 * b + hh]

        # paired full k-tiles -> [128, 1024] psum -> one wide exp
        rhs_slices = []  # (P tile, col offset or diag marker) per ki
        for kj in range(2 * qc):
            spb = psA.tile([128, 1024], F32, name="psA")
            for half in range(2):
                ki = 2 * kj + half
                nc.tensor.matmul(
                    spb[:, 512 * half : 512 * (half + 1)],
                    lhsT=kh[:, 128 * ki : 128 * (ki + 1)],
                    rhs=qh,
                    start=True,
                    stop=True,
                )
            Pb = pbig.tile([128, 1024], BF16, name="Pbig")
            nc.scalar.activation(Pb[:], spb[:], EXPF, scale=SCALE)
            rhs_slices.append((Pb, 0))
            rhs_slices.append((Pb, 512))
        # diagonal k-tiles
        for ki in range(4 * qc, nk):
            spd = psB.tile([128, 512], F32, name="psB")
            nc.tensor.matmul(
                spd[:],
                lhsT=kh[:, 128 * ki : 128 * (ki + 1)],
                rhs=qh,
                start=True,
                stop=True,
            )
            Pd = pdiag.tile([128, 512], BF16, name="Pdiag")
            r = 128 * ki - 512 * qc
            nc.scalar.activation(Pd[:, r:512], spd[:, r:512], EXPF, scale=SCALE)
            nc.gpsimd.affine_select(
                out=Pd[:, r:512],
                in_=Pd[:, r:512],
                compare_op=mybir.AluOpType.is_ge,
                fill=0.0,
                base=0,
                pattern=[[1, 512 - r]],
                channel_multiplier=-1,
            )
            rhs_slices.append((Pd, r - 512))  # negative marks diag offset r

        op = psC.tile([128, 512], F32, name="psC")
        for ki in range(nk):
            Pt, off = rhs_slices[ki]
            if off >= 0:
                rhs = Pt[:, off : off + 512]
                r = 0
            else:
                r = off + 512
                rhs = Pt[:, r:512]
            # M=128 windows read 63 cols into chunk ki+1, which for the last
            # diag chunk isn't produced yet -> stop at the chunk edge there.
            m = 65 if ki == nk - 1 else 128
            nc.tensor.matmul(
                op[0:m, r:512],
                lhsT=v1[:, 65 * ki : 65 * ki + m],
                rhs=rhs,
                start=(ki == 0),
                stop=(ki == nk - 1),
                skip_group_check=True,
            )
        oT = otpool.tile([65, 512], BF16, name="oT")
        nc.vector.tensor_copy(oT[:], op[0:65, :])

        tr = psD.tile([128, 272], BF16, name="psD_tr", tag="psD")
        for j in range(4):
            nc.tensor.transpose(
                tr[:, 256 + 4 * j : 257 + 4 * j],
                oT[64:65, 128 * j : 128 * (j + 1)],
                ident_b[64:65, 64:65],
            )
            nc.tensor.transpose(
                tr[:, 64 * j : 64 * (j + 1)],
                oT[0:64, 128 * j : 128 * (j + 1)],
                ident_b[0:64, 0:64],
            )
        rc = rpool.tile([128, 4], F32, name="rc")
        den4 = tr[:, 256:272].rearrange("p (a b) -> p a b", b=4)[:, :, 0]
        nc.vector.reciprocal(rc[:], den4)
        for j in range(4):
            col = 128 * (4 * qc + j) + 64 * hh
            nc.vector.tensor_scalar_mul(
                outsb[b][:, col : col + 64],
                tr[:, 64 * j : 64 * (j + 1)],
                rc[:, j : j + 1],
            )

    # ---- pipelined stages: 512-token proj chunk -> K/V piece -> q-chunk ----
    for st in range(8):  # b = st // 4, qc = st % 4
        b, qc = st // 4, st % 4
        t0 = 512 * st
        xts = []
        for cc in range(8):
            xt = xpool.tile([128, 512], BF16, name="xt")
            eng = nc.sync if cc % 2 == 0 else nc.scalar
            eng.dma_start(
                xt[:], xT_ap[128 * cc : 128 * (cc + 1), t0 : t0 + 512]
            )
            xts.append(xt)
        for nt in range(3):
            ps = psA.tile([128, 1024], F32, name="psA")
            for cc in range(8):
                nc.tensor.matmul(
                    ps[:, 0:512],
                    lhsT=W8[cc][:, 128 * nt : 128 * (nt + 1)],
                    rhs=xts[cc][:],
                    start=(cc == 0),
                    stop=(cc == 7),
                )
            nc.vector.tensor_scalar_add(
                qkvT[nt][:, t0 : t0 + 512], ps[:, 0:512], bias[:, nt : nt + 1]
            )

        # K^T zero-padded piece (this stage's 512 tokens)
        lo = 512 * qc
        nc.vector.tensor_copy(
            kpad[b][0][0:64, lo : lo + 512], qkvT[1][0:64, t0 : t0 + 512]
        )
        nc.vector.tensor_copy(
            kpad[b][1][64:128, lo : lo + 512], qkvT[1][64:128, t0 : t0 + 512]
        )
        # V natural piece: k-chunks 4*qc .. 4*qc+3
        for hh in range(2):
            v3 = V1[2 * b + hh][:].rearrange("p (k u) -> p k u", u=65)
            tp = psD.tile([128, 256], BF16, name="psD_vt", tag="psD")
            for j in range(4):
                kc = 4 * qc + j
                nc.tensor.transpose(
                    tp[:, 64 * j : 64 * (j + 1)],
                    qkvT[2][64 * hh : 64 * (hh + 1), 2048 * b + 128 * kc : 2048 * b + 128 * (kc + 1)],
                    ident_b[64 * hh : 64 * hh + 64, 64 * hh : 64 * hh + 64],
                )
            nc.vector.tensor_copy(
                v3[:, 4 * qc : 4 * (qc + 1), 0:64],
                tp[:].rearrange("p (k u) -> p k u", u=64),
            )

        for hh in range(2):
            attention_unit(b, hh, qc)
        # this (b, qc) row block is complete: stream it out
        dst = out_ap[2048 * b + 512 * qc : 2048 * b + 512 * (qc + 1), :]
        dst = dst.rearrange("(k p) c -> p k c", p=128)
        src = outsb[b][:, 512 * qc : 512 * (qc + 1)]
        src = src.rearrange("p (k c) -> p k c", c=128)
        nc.sync.dma_start(dst, src)

def build_program():
    nc = bacc.Bacc("TRN2", target_bir_lowering=False, debug=False, num_devices=NCORES)
    xT_h = nc.dram_tensor("xT", [C, TOK], BF16, kind="ExternalInput").ap()
    w_h = nc.dram_tensor("w", [C, N3], BF16, kind="ExternalInput").ap()
    b_h = nc.dram_tensor("b", [128, 19], F32, kind="ExternalInput").ap()
    out_h = nc.dram_tensor("out", [TOK, CS], F32, kind="ExternalOutput").ap()
    with tile.TileContext(nc) as tc:
        mha_kernel(tc, out_h, xT_h, w_h, b_h)
    nc.compile()
    return nc


def make_in_maps(x, w_qkv, b_qkv):
    x = np.asarray(x, dtype=np.float32)
    w_qkv = np.asarray(w_qkv, dtype=np.float32)
    b_qkv = np.asarray(b_qkv, dtype=np.float32)
    xT = np.ascontiguousarray(x.reshape(TOK, C).T).astype(ml_dtypes.bfloat16)
    in_maps = []
    for c in range(NCORES):
        sl = slice(CS * c, CS * (c + 1))
        w_c = np.ascontiguousarray(
            np.concatenate(
                [w_qkv[:, sl], w_qkv[:, C + CS * c : C + CS * (c + 1)],
                 w_qkv[:, 2 * C + CS * c : 2 * C + CS * (c + 1)]],
                axis=1,
            )
        ).astype(ml_dtypes.bfloat16)
        b_c = np.concatenate(
            [b_qkv[sl], b_qkv[C + CS * c : C + CS * (c + 1)],
             b_qkv[2 * C + CS * c : 2 * C + CS * (c + 1)]]
        )
        b_c = b_c.reshape(3, 128).T  # [128, 3]
        b_c = np.ascontiguousarray(
            np.concatenate([b_c, np.ones((128, 16), np.float32)], axis=1)
        )  # [128, 19]
        in_maps.append({"xT": xT, "w": w_c, "b": b_c})
    return in_maps


_NC_CACHE = None


def kernel(x, w_qkv, b_qkv):
    global _NC_CACHE
    if _NC_CACHE is None:
        _NC_CACHE = build_program()
    nc = _NC_CACHE
    in_maps = make_in_maps(x, w_qkv, b_qkv)
    res = run_bass_kernel_spmd(nc, in_maps, list(range(NCORES)))
    outs = [res.results[c]["out"].reshape(B, T, CS) for c in range(NCORES)]
    return np.concatenate(outs, axis=2)


if __name__ == "__main__":
    rng = np.random.default_rng(0)
    x = rng.standard_normal((B, T, C), dtype=np.float32)
    w = (rng.standard_normal((C, 3 * C), dtype=np.float32) / np.sqrt(C)).astype(np.float32)
    bq = (rng.standard_normal((3 * C,), dtype=np.float32) * 0.02).astype(np.float32)
    out = kernel(x, w, bq)
    print("out", out.shape, out.dtype)


# revision 17
# speedup vs baseline: 83.3564x; 1.0903x over previous
"""Multi-head causal attention (B=2, T=2048, C=1024, H=16, D=64) on 8 TRN2
NeuronCores, tensor-parallel over heads: core c owns heads {2c, 2c+1}.

Per-core program (SPMD, same code, per-core weight slices), all matmuls bf16
with fp32 PSUM accumulation. Pipelined in 4 stages of 1024 tokens each:
projection chunk -> V-transposes + zero-padded K^T piece -> the attention
q-chunks whose causal window is now complete. This keeps TensorE dense
(HAM stays at full clock) and lets ScalarE exp overlap the next projection.

Attention per (b, head, 512-wide q-chunk):
  s^T[k, q] = Kpad^T.T @ Q^T  (K=128 via zero-padding, full-array matmuls;
  two k-tiles share a 2-bank psum -> one 1024-wide exp), causal diagonal
  tiles get range-limited exp + gpsimd affine_select zeroing,
  outT[d,q] + denom row = [V | 1].T @ P^T with M=128 overlapping lhsT
  windows, PE-transpose back to [q, d] (bf16), scale by 1/denom on DVE.
Host: transposes x, converts x/W to bf16, slices W/b per core, concatenates
per-core [4096, 128] outputs on channels.
"""
import sys

if "/opt/trn_rl_repo" not in sys.path:
    sys.path.insert(0, "/opt/trn_rl_repo")

from contextlib import ExitStack

import numpy as np
import ml_dtypes

import concourse.bass as bass
import concourse.tile as tile
from concourse import bacc, mybir
from concourse._compat import with_exitstack
from concourse.bass_utils import run_bass_kernel_spmd
from concourse.masks import make_identity

F32 = mybir.dt.float32
BF16 = mybir.dt.bfloat16
EXPF = mybir.ActivationFunctionType.Exp

B, T, C = 2, 2048, 1024
H, D = 16, 64
NCORES = 8
TOK = B * T            # 4096
CS = 128               # channel slice per core (2 heads x 64)
N3 = 3 * CS            # 384 qkv columns per core
SCALE = 1.0 / np.sqrt(D)


@with_exitstack
def mha_kernel(ctx: ExitStack, tc: tile.TileContext, out_ap, xT_ap, w_ap, b_ap):
    nc = tc.nc

    cst = ctx.enter_context(tc.tile_pool(name="cst", bufs=1))
    xpool = ctx.enter_context(tc.tile_pool(name="xt", bufs=10))
    pbig = ctx.enter_context(tc.tile_pool(name="pbig", bufs=10))
    pdiag = ctx.enter_context(tc.tile_pool(name="pdiag", bufs=8))
    otpool = ctx.enter_context(tc.tile_pool(name="ot", bufs=3))
    rpool = ctx.enter_context(tc.tile_pool(name="rc", bufs=3))
    psA = ctx.enter_context(tc.tile_pool(name="psA", bufs=2, space="PSUM"))  # big: proj + paired scores
    psB = ctx.enter_context(tc.tile_pool(name="psB", bufs=2, space="PSUM"))  # diag scores
    psC = ctx.enter_context(tc.tile_pool(name="psC", bufs=1, space="PSUM"))  # pv
    psD = ctx.enter_context(tc.tile_pool(name="psD", bufs=1, space="PSUM"))  # transposes

    ident_f = cst.tile([128, 128], F32, name="ident_f")
    make_identity(nc, ident_f[:])
    ident_b = cst.tile([128, 128], BF16, name="ident_b")
    nc.vector.tensor_copy(ident_b[:], ident_f[:])

    # bias cols 0-2; cols 3-18 are ones (for the PV denominator column)
    bias = cst.tile([128, 19], F32, name="bias")
    nc.sync.dma_start(bias[:], b_ap[:])

    W8 = []
    for cc in range(8):
        w = cst.tile([128, N3], BF16, name=f"w{cc}")
        nc.sync.dma_start(w[:], w_ap[128 * cc : 128 * (cc + 1), :])
        W8.append(w)

    qkvT = [cst.tile([128, TOK], BF16, name=f"qkvT{nt}") for nt in range(3)]

    # persistent attention tensors
    # kpad[b][0]: rows 0-63 = K^T_h0, rows 64-127 = 0
    # kpad[b][1]: rows 0-63 = 0,      rows 64-127 = K^T_h1
    kpad = []
    for b in range(B):
        k0t = cst.tile([128, 2048], BF16, name=f"kpad0_{b}")
        nc.vector.memset(k0t[64:128, :], 0.0)
        k1t = cst.tile([128, 2048], BF16, name=f"kpad1_{b}")
        nc.vector.memset(k1t[0:64, :], 0.0)
        kpad.append((k0t, k1t))
    V1 = []
    for b in range(B):
        for hh in range(2):
            v1 = cst.tile([128, 16 * 65], BF16, name=f"v1_{b}_{hh}")
            v3 = v1[:].rearrange("p (k u) -> p k u", u=65)
            nc.vector.tensor_copy(v3[:, :, 64], bias[:, 3:19])
            V1.append(v1)
    outsb = [cst.tile([128, 2048], F32, name=f"outsb{b}") for b in range(B)]

    def attention_unit(b, hh, qc):
        q0 = 2048 * b + 512 * qc
        nk = 4 * qc + 4
        kh = kpad[b][hh]
        qh = qkvT[0][:, q0 : q0 + 512]
        v1 = V1[2 * b + hh]

        # paired full k-tiles -> [128, 1024] psum -> one wide exp
        rhs_slices = []  # (P tile, col offset or diag marker) per ki
        for kj in range(2 * qc):
            spb = psA.tile([128, 1024], F32, name="psA")
            for half in range(2):
                ki = 2 * kj + half
                nc.tensor.matmul(
                    spb[:, 512 * half : 512 * (half + 1)],
                    lhsT=kh[:, 128 * ki : 128 * (ki + 1)],
                    rhs=qh,
                    start=True,
                    stop=True,
                )
            Pb = pbig.tile([128, 1024], BF16, name="Pbig")
            nc.scalar.activation(Pb[:], spb[:], EXPF, scale=SCALE)
            rhs_slices.append((Pb, 0))
            rhs_slices.append((Pb, 512))
        # diagonal k-tiles
        for ki in range(4 * qc, nk):
            spd = psB.tile([128, 512], F32, name="psB")
            r = 128 * ki - 512 * qc
            nc.tensor.matmul(
                spd[:, r:512],
                lhsT=kh[:, 128 * ki : 128 * (ki + 1)],
                rhs=qh[:, r:512],
                start=True,
                stop=True,
            )
            Pd = pdiag.tile([128, 512], BF16, name="Pdiag")
            nc.scalar.activation(Pd[:, r:512], spd[:, r:512], EXPF, scale=SCALE)
            nc.gpsimd.affine_select(
                out=Pd[:, r:512],
                in_=Pd[:, r:512],
                compare_op=mybir.AluOpType.is_ge,
                fill=0.0,
                base=0,
                pattern=[[1, 512 - r]],
                channel_multiplier=-1,
            )
            rhs_slices.append((Pd, r - 512))  # negative marks diag offset r

        op = psC.tile([128, 512], F32, name="psC")
        for ki in range(nk):
            Pt, off = rhs_slices[ki]
            if off >= 0:
                rhs = Pt[:, off : off + 512]
                r = 0
            else:
                r = off + 512
                rhs = Pt[:, r:512]
            # M=128 windows read 63 cols into chunk ki+1, which for the last
            # diag chunk isn't produced yet -> stop at the chunk edge there.
            m = 65 if ki == nk - 1 else 128
            nc.tensor.matmul(
                op[0:m, r:512],
                lhsT=v1[:, 65 * ki : 65 * ki + m],
                rhs=rhs,
                start=(ki == 0),
                stop=(ki == nk - 1),
                skip_group_check=True,
            )
        oT = otpool.tile([65, 512], BF16, name="oT")
        nc.vector.tensor_copy(oT[:], op[0:65, :])

        tr = psD.tile([128, 264], BF16, name="psD_tr", tag="psD")
        for j in range(4):
            nc.tensor.transpose(
                tr[:, 66 * j : 66 * j + 65],
                oT[0:65, 128 * j : 128 * (j + 1)],
                ident_b[0:65, 0:65],
            )
        rc = rpool.tile([128, 4], F32, name="rc")
        den4 = tr[:].rearrange("p (a b) -> p a b", b=66)[:, :, 64]
        nc.vector.reciprocal(rc[:], den4)
        for j in range(4):
            col = 128 * (4 * qc + j) + 64 * hh
            nc.vector.tensor_scalar_mul(
                outsb[b][:, col : col + 64],
                tr[:, 66 * j : 66 * j + 64],
                rc[:, j : j + 1],
            )

    # ---- pipelined stages: 512-token proj chunk -> K/V piece -> q-chunk ----
    for st in range(8):  # b = st // 4, qc = st % 4
        b, qc = st // 4, st % 4
        t0 = 512 * st
        xts = []
        for cc in range(8):
            xt = xpool.tile([128, 512], BF16, name="xt")
            eng = nc.sync if cc % 2 == 0 else nc.scalar
            eng.dma_start(
                xt[:], xT_ap[128 * cc : 128 * (cc + 1), t0 : t0 + 512]
            )
            xts.append(xt)
        for nt in range(3):
            ps = psA.tile([128, 1024], F32, name="psA")
            for cc in range(8):
                nc.tensor.matmul(
                    ps[:, 0:512],
                    lhsT=W8[cc][:, 128 * nt : 128 * (nt + 1)],
                    rhs=xts[cc][:],
                    start=(cc == 0),
                    stop=(cc == 7),
                )
            nc.vector.tensor_scalar_add(
                qkvT[nt][:, t0 : t0 + 512], ps[:, 0:512], bias[:, nt : nt + 1]
            )

        # K^T zero-padded piece (this stage's 512 tokens)
        lo = 512 * qc
        nc.vector.tensor_copy(
            kpad[b][0][0:64, lo : lo + 512], qkvT[1][0:64, t0 : t0 + 512]
        )
        nc.vector.tensor_copy(
            kpad[b][1][64:128, lo : lo + 512], qkvT[1][64:128, t0 : t0 + 512]
        )
        # V natural piece: k-chunks 4*qc .. 4*qc+3
        for hh in range(2):
            v3 = V1[2 * b + hh][:].rearrange("p (k u) -> p k u", u=65)
            tp = psD.tile([128, 256], BF16, name="psD_vt", tag="psD")
            for j in range(4):
                kc = 4 * qc + j
                nc.tensor.transpose(
                    tp[:, 64 * j : 64 * (j + 1)],
                    qkvT[2][64 * hh : 64 * (hh + 1), 2048 * b + 128 * kc : 2048 * b + 128 * (kc + 1)],
                    ident_b[64 * hh : 64 * hh + 64, 64 * hh : 64 * hh + 64],
                )
            nc.vector.tensor_copy(
                v3[:, 4 * qc : 4 * (qc + 1), 0:64],
                tp[:].rearrange("p (k u) -> p k u", u=64),
            )

        for hh in range(2):
            attention_unit(b, hh, qc)
        # this (b, qc) row block is complete: stream it out
        dst = out_ap[2048 * b + 512 * qc : 2048 * b + 512 * (qc + 1), :]
        dst = dst.rearrange("(k p) c -> p k c", p=128)
        src = outsb[b][:, 512 * qc : 512 * (qc + 1)]
        src = src.rearrange("p (k c) -> p k c", c=128)
        nc.sync.dma_start(dst, src)

def build_program():
    nc = bacc.Bacc("TRN2", target_bir_lowering=False, debug=False, num_devices=NCORES)
    xT_h = nc.dram_tensor("xT", [C, TOK], BF16, kind="ExternalInput").ap()
    w_h = nc.dram_tensor("w", [C, N3], BF16, kind="ExternalInput").ap()
    b_h = nc.dram_tensor("b", [128, 19], F32, kind="ExternalInput").ap()
    out_h = nc.dram_tensor("out", [TOK, CS], F32, kind="ExternalOutput").ap()
    with tile.TileContext(nc) as tc:
        mha_kernel(tc, out_h, xT_h, w_h, b_h)
    nc.compile()
    return nc


def make_in_maps(x, w_qkv, b_qkv):
    x = np.asarray(x, dtype=np.float32)
    w_qkv = np.asarray(w_qkv, dtype=np.float32)
    b_qkv = np.asarray(b_qkv, dtype=np.float32)
    xT = np.ascontiguousarray(x.reshape(TOK, C).T).astype(ml_dtypes.bfloat16)
    in_maps = []
    for c in range(NCORES):
        sl = slice(CS * c, CS * (c + 1))
        w_c = np.ascontiguousarray(
            np.concatenate(
                [w_qkv[:, sl], w_qkv[:, C + CS * c : C + CS * (c + 1)],
                 w_qkv[:, 2 * C + CS * c : 2 * C + CS * (c + 1)]],
                axis=1,
            )
        ).astype(ml_dtypes.bfloat16)
        b_c = np.concatenate(
            [b_qkv[sl], b_qkv[C + CS * c : C + CS * (c + 1)],
             b_qkv[2 * C + CS * c : 2 * C + CS * (c + 1)]]
        )
        b_c = b_c.reshape(3, 128).T  # [128, 3]
        b_c = np.ascontiguousarray(
            np.concatenate([b_c, np.ones((128, 16), np.float32)], axis=1)
        )  # [128, 19]
        in_maps.append({"xT": xT, "w": w_c, "b": b_c})
    return in_maps


_NC_CACHE = None


def kernel(x, w_qkv, b_qkv):
    global _NC_CACHE
    if _NC_CACHE is None:
        _NC_CACHE = build_program()
    nc = _NC_CACHE
    in_maps = make_in_maps(x, w_qkv, b_qkv)
    res = run_bass_kernel_spmd(nc, in_maps, list(range(NCORES)))
    outs = [res.results[c]["out"].reshape(B, T, CS) for c in range(NCORES)]
    return np.concatenate(outs, axis=2)


if __name__ == "__main__":
    rng = np.random.default_rng(0)
    x = rng.standard_normal((B, T, C), dtype=np.float32)
    w = (rng.standard_normal((C, 3 * C), dtype=np.float32) / np.sqrt(C)).astype(np.float32)
    bq = (rng.standard_normal((3 * C,), dtype=np.float32) * 0.02).astype(np.float32)
    out = kernel(x, w, bq)
    print("out", out.shape, out.dtype)
